# revision 12
# baseline (speedup 1.0000x reference)
"""AdaptiveRankLSTM Trainium2 kernel.

Math: the TT cores compose into rank-16 linear maps:
  W_ih = Ai_s @ Ai_e  (1024x16 @ 16x4096),  W_hh = Ah_s @ Ah_e.
Recurrence per step:  gates = x_t @ W_ih + h @ W_hh + bias
 -> precompute sqi = x @ Ai_s for all t (phase 1),
    per step: s_hh = h @ Ah_s (16 dims), gates = [sqi_t; s_hh] @ A_cat + bias.

Device layout: everything transposed (feature dims on partitions).
  hT/cT: (128, 64) cols = j*8+b  (j = H-tile of 128, b = batch row)
  gatesT: (128, 256) cols = gtile*8+b, gate-tile order [i(0:8) f(8:16) o(16:24) g(24:32)]
Sharding: batch 64 -> 8 cores x 8 rows (data parallel, recurrence local).
"""
import os
import numpy as np

B, T, D, H, R = 64, 512, 1024, 1024, 16
NCORES = 8
BS = B // NCORES          # 8 batch rows per core
NS = BS * T               # 4096 samples per core
NJ = H // 128             # 8 H-tiles
NG = 4 * H // 128         # 32 gate tiles
f32 = np.float32

_cache = {}


def _compose_host(ih, hh, b_ih, b_hh):
    """Build squeeze mats (1024,16), packed cat-expansion (8,128,128), biasT."""
    def sq(g1, g2, g3):
        return np.einsum('ia,ajc,ckd->ijkd', g1, g2, g3).reshape(D, R).astype(f32)

    def ex(g4, g5, g6):
        return np.einsum('dne,emf,fp->dnmp', g4, g5, g6).reshape(R, 4 * H).astype(f32)

    Ai_s, Ai_e = sq(*ih[:3]), ex(*ih[3:])
    Ah_s, Ah_e = sq(*hh[:3]), ex(*hh[3:])
    bias = (np.asarray(b_ih) + np.asarray(b_hh)).astype(f32)

    # reorder gate columns i,f,g,o -> i,f,o,g
    perm = np.concatenate([np.arange(0, H), np.arange(H, 2 * H),
                           np.arange(3 * H, 4 * H), np.arange(2 * H, 3 * H)])
    A_cat = np.concatenate([Ai_e, Ah_e], axis=0)[:, perm]   # (32, 4096)
    bias = bias[perm]

    # squeeze mats packed: (128, 16j + a) per k-tile j
    def pack_sq(A):  # (1024,16) -> (128, 128)
        return A.reshape(NJ, 128, R).transpose(1, 0, 2).reshape(128, NJ * R).copy()

    wsq = pack_sq(Ai_s)  # (128, 128) for phase 1

    # phase-2 "repl column" builders: 9 stationaries producing rp (128, 8) =
    # per q-block of 32 rows: rows 0:16 = sqi_t, rows 16:32 = s_hh.
    # wrep block 0 = E: E[a, 32q+a'] = (a'==a), K=16 (applied to sqiT slice)
    # wrep block 1+j = Ah_s tile j replicated: [k, 32q+16+a] = Ah_s[j*128+k, a]
    wrep = np.zeros((128, 9 * 128), f32)
    for q in range(4):
        wrep[:R, 32 * q:32 * q + R] = np.eye(R, dtype=f32)
        for j in range(NJ):
            wrep[:, (1 + j) * 128 + 32 * q + R:(1 + j) * 128 + 32 * q + 2 * R] = \
                Ah_s[j * 128:(j + 1) * 128, :]

    # expansion stationaries: 8 groups of (128,128); group G rows 32q+k,
    # col m = A_cat[k, (G*4+q)*128 + m]
    wex = np.zeros((128, 8 * 128), f32)
    for G in range(8):
        for q in range(4):
            gt = G * 4 + q
            wex[32 * q:32 * q + 32, G * 128:(G + 1) * 128] = \
                A_cat[:, gt * 128:(gt + 1) * 128]

    biasT = bias.reshape(NG, 128).T.copy()  # (128, 32)
    return wsq, wrep, wex, biasT


def _build_nc():
    import concourse.bacc as bacc
    import concourse.mybir as mybir
    import concourse.tile as tile
    
    dt = mybir.dt.float32
    nc = bacc.Bacc()
    x_d = nc.dram_tensor("xT", (D, NS), dt, kind="ExternalInput")
    wsq_d = nc.dram_tensor("wsq", (128, NJ * R), dt, kind="ExternalInput")
    wrep_d = nc.dram_tensor("wrep", (128, 9 * 128), dt, kind="ExternalInput")
    wex_d = nc.dram_tensor("wex", (128, 8 * 128), dt, kind="ExternalInput")
    biasT_d = nc.dram_tensor("biasT", (128, NG), dt, kind="ExternalInput")
    out_d = nc.dram_tensor("out", (T, NJ, BS, 128), dt, kind="ExternalOutput")
    c_d = nc.dram_tensor("c_out", (NJ, BS, 128), dt, kind="ExternalOutput")

    with tile.TileContext(nc) as tc:
        with (
            tc.tile_pool(name="consts", bufs=1) as consts,
            tc.tile_pool(name="state", bufs=1) as state,
            tc.tile_pool(name="xload", bufs=8) as xload,
            tc.tile_pool(name="work", bufs=3) as work,
            tc.tile_pool(name="ps_sq", bufs=2, space="PSUM") as ps_sq,
            tc.tile_pool(name="ps_g", bufs=2, space="PSUM") as ps_g,
        ):
            wsq0 = consts.tile([128, NJ * R], dt)
            nc.sync.dma_start(out=wsq0, in_=wsq_d[:, :])
            wsq = consts.tile([128, NJ * R], dt)
            nc.vector.tensor_copy(wsq, wsq0)
            wrep0 = consts.tile([128, 9 * 128], dt)
            nc.sync.dma_start(out=wrep0, in_=wrep_d[:, :])
            wrep = consts.tile([128, 9 * 128], dt)
            nc.vector.tensor_copy(wrep, wrep0)
            wex0 = consts.tile([128, 8 * 128], dt)
            nc.sync.dma_start(out=wex0, in_=wex_d[:, :])
            wex = consts.tile([128, 8 * 128], dt)
            nc.vector.tensor_copy(wex, wex0)
            biasT = consts.tile([128, NG], dt)
            nc.sync.dma_start(out=biasT, in_=biasT_d[:, :])

            sqiT = consts.tile([R, NS], dt)          # (16, 4096) t-major cols t*8+b
            hT = state.tile([128, NJ * BS], dt)
            cT = state.tile([128, NJ * BS], dt)
            repl = state.tile([128, 4 * BS], dt)     # block-diag rhs for expansion
            nc.vector.memset(hT, 0.0)
            nc.vector.memset(cT, 0.0)
            nc.vector.memset(repl, 0.0)

            # ---------------- phase 1: sqiT = Ai_s^T @ xT, t-major ----------
            for sl in range(8):
                xs = xload.tile([128, NJ, 512], dt)
                src = x_d.rearrange("(j p) n -> p j n", p=128)
                nc.sync.dma_start(out=xs, in_=src[:, :, sl * 512:(sl + 1) * 512])
                sq_ps = ps_sq.tile([R, 512], dt, tag="sq1")
                for j in range(NJ):
                    nc.tensor.matmul(sq_ps, wsq[:, j * R:(j + 1) * R],
                                     xs[:, j, :],
                                     start=(j == 0), stop=(j == NJ - 1))
                nc.vector.tensor_copy(sqiT[:, sl * 512:(sl + 1) * 512], sq_ps)

            # ---------------- phase 2: recurrence ---------------------------
            for t in range(T):
                # rp (128,8): q-block rows 32q:32q+16 = sqi_t, +16:+32 = s_hh
                rp_ps = ps_sq.tile([128, BS], dt, tag="sq")
                nc.tensor.matmul(rp_ps, wrep[:R, 0:128],
                                 sqiT[:, t * BS:(t + 1) * BS],
                                 start=True, stop=False)
                for j in range(NJ):
                    nc.tensor.matmul(rp_ps, wrep[:, (1 + j) * 128:(2 + j) * 128],
                                     hT[:, j * BS:(j + 1) * BS],
                                     start=False, stop=(j == NJ - 1))
                # scatter into block-diag rhs (32-aligned partition copies)
                for q in range(4):
                    nc.vector.tensor_copy(
                        repl[32 * q:32 * q + 32, q * BS:(q + 1) * BS],
                        rp_ps[32 * q:32 * q + 32, :])
                g_ps = ps_g.tile([128, NG * BS], dt, tag="gates")
                for G in range(8):
                    nc.tensor.matmul(g_ps[:, G * 32:(G + 1) * 32],
                                     wex[:, G * 128:(G + 1) * 128], repl,
                                     start=True, stop=True)
                gates = work.tile([128, NG * BS], dt, tag="gates_sb")
                # bias add (broadcast over batch) psum -> sbuf
                nc.vector.tensor_add(
                    gates.rearrange("p (g b) -> p g b", b=BS),
                    g_ps.rearrange("p (g b) -> p g b", b=BS),
                    biasT.rearrange("p (g o) -> p g o", o=1).to_broadcast([128, NG, BS]))
                sig = work.tile([128, 24 * BS], dt, tag="sig")
                nc.scalar.activation(sig, gates[:, :24 * BS],
                                     mybir.ActivationFunctionType.Sigmoid)
                tg = work.tile([128, NJ * BS], dt, tag="tg")
                nc.scalar.activation(tg, gates[:, 24 * BS:],
                                     mybir.ActivationFunctionType.Tanh)
                # c = sig_f * c + sig_i * tg
                t1 = work.tile([128, NJ * BS], dt, tag="t1")
                nc.vector.tensor_mul(t1, sig[:, :NJ * BS], tg)
                nc.vector.tensor_mul(cT, sig[:, NJ * BS:2 * NJ * BS], cT)
                nc.vector.tensor_add(cT, cT, t1)
                tc_t = work.tile([128, NJ * BS], dt, tag="tc")
                nc.scalar.activation(tc_t, cT, mybir.ActivationFunctionType.Tanh)
                nc.vector.tensor_mul(hT, sig[:, 2 * NJ * BS:3 * NJ * BS], tc_t)
                # stream h_t to DRAM: out[b, t, j*128+p] = hT[p, j*8+b]
                dst = out_d[t].rearrange("j b p -> p j b")
                nc.sync.dma_start(out=dst, in_=hT.rearrange("p (j b) -> p j b", b=BS))

            dstc = c_d.rearrange("j b p -> p j b")
            nc.sync.dma_start(out=dstc, in_=cT.rearrange("p (j b) -> p j b", b=BS))
    nc.compile()
    return nc


def kernel(x, ih_g1, ih_g2, ih_g3, ih_g4, ih_g5, ih_g6,
           hh_g1, hh_g2, hh_g3, hh_g4, hh_g5, hh_g6, b_ih, b_hh):
    from concourse.bass_utils import run_bass_kernel_spmd

    x = np.asarray(x, dtype=f32)
    wsq, wrep, wex, biasT = _compose_host(
        [np.asarray(g, f32) for g in (ih_g1, ih_g2, ih_g3, ih_g4, ih_g5, ih_g6)],
        [np.asarray(g, f32) for g in (hh_g1, hh_g2, hh_g3, hh_g4, hh_g5, hh_g6)],
        b_ih, b_hh)

    if "nc" not in _cache:
        _cache["nc"] = _build_nc()
    nc = _cache["nc"]

    in_maps = []
    for c in range(NCORES):
        xs = np.ascontiguousarray(
            x[c * BS:(c + 1) * BS].transpose(2, 1, 0).reshape(D, NS))
        in_maps.append({"xT": xs, "wsq": wsq, "wrep": wrep, "wex": wex,
                        "biasT": biasT})

    trace = bool(int(os.environ.get("KBENCH_TRACE", "0")))
    res = run_bass_kernel_spmd(nc, in_maps, core_ids=list(range(NCORES)),
                               trace=trace)
    if trace:
        print(f"HW exec time: {res.exec_time_ns} ns")
        if res.instructions_and_trace:
            print("trace:", res.instructions_and_trace[1])

    out = np.empty((B, T, H), f32)
    c_fin = np.empty((B, H), f32)
    for c in range(NCORES):
        o = res.results[c]["out"]           # (T, NJ, BS, 128)
        out[c * BS:(c + 1) * BS] = o.transpose(2, 0, 1, 3).reshape(BS, T, H)
        c_fin[c * BS:(c + 1) * BS] = res.results[c]["c_out"].transpose(1, 0, 2).reshape(BS, H)
    h_fin = np.ascontiguousarray(out[:, -1, :])
    return out, (h_fin, c_fin)


# revision 24
# speedup vs baseline: 56.2377x; 56.2377x over previous
"""AdaptiveRankLSTM Trainium2 kernel.

Math: the TT cores compose into rank-16 linear maps:
  W_ih = Ai_s @ Ai_e  (1024x16 @ 16x4096),  W_hh = Ah_s @ Ah_e.
Recurrence per step:  gates = x_t @ W_ih + h @ W_hh + bias
 -> precompute sqi = x @ Ai_s for all t (phase 1),
    per step: s_hh = h @ Ah_s (16 dims), gates = [sqi_t; s_hh] @ A_cat + bias.

Device layout: everything transposed (feature dims on partitions).
  hT/cT: (128, 64) cols = j*8+b  (j = H-tile of 128, b = batch row)
  gatesT: (128, 256) cols = gtile*8+b, gate-tile order [i(0:8) f(8:16) o(16:24) g(24:32)]
Sharding: batch 64 -> 8 cores x 8 rows (data parallel, recurrence local).
"""
import os
import numpy as np
import ml_dtypes

B, T, D, H, R = 64, 512, 1024, 1024, 16
NCORES = 8
BS = B // NCORES          # 8 batch rows per core
NS = BS * T               # 4096 samples per core
NJ = H // 128             # 8 H-tiles
NG = 4 * H // 128         # 32 gate tiles
f32 = np.float32

_cache = {}


def _compose_host(ih, hh, b_ih, b_hh):
    """Build squeeze mats (1024,16), packed cat-expansion (8,128,128), biasT."""
    def sq(g1, g2, g3):
        return np.einsum('ia,ajc,ckd->ijkd', g1, g2, g3).reshape(D, R).astype(f32)

    def ex(g4, g5, g6):
        return np.einsum('dne,emf,fp->dnmp', g4, g5, g6).reshape(R, 4 * H).astype(f32)

    Ai_s, Ai_e = sq(*ih[:3]), ex(*ih[3:])
    Ah_s, Ah_e = sq(*hh[:3]), ex(*hh[3:])
    bias = (np.asarray(b_ih) + np.asarray(b_hh)).astype(f32)

    # reorder gate columns i,f,g,o -> i,f,o,g
    perm = np.concatenate([np.arange(0, H), np.arange(H, 2 * H),
                           np.arange(3 * H, 4 * H), np.arange(2 * H, 3 * H)])
    # contraction row order per 32-block: rows 0:16 = Ah_e (s_hh), 16:32 = Ai_e
    A_cat = np.concatenate([Ah_e, Ai_e], axis=0)[:, perm]   # (32, 4096)
    bias = bias[perm]

    # squeeze mats packed: (128, 16j + a) per k-tile j
    def pack_sq(A):  # (1024,16) -> (128, 128)
        return A.reshape(NJ, 128, R).transpose(1, 0, 2).reshape(128, NJ * R).copy()

    # phase-1 squeeze stationaries, M=128: out rows 32q+16+a = sqi[a]
    # (rows 32q:32q+16 stay zero -- the s_hh slots)
    wsq = np.zeros((NJ, 128, 128), f32)
    for q in range(4):
        for j in range(NJ):
            wsq[j, :, 32 * q + R:32 * q + 2 * R] = Ai_s[j * 128:(j + 1) * 128, :]
    wsq = wsq.transpose(1, 0, 2).reshape(128, NJ * 128).copy()

    # phase-2 squeeze: Ah_s tiles, M=16 (cols 16j+a)
    wrep = pack_sq(Ah_s)  # (128, 128)

    # expansion stationaries: 8 groups of (128,128); group G rows 32q+k,
    # col m = A_cat[k, (G*4+q)*128 + m]
    wex = np.zeros((128, 8 * 128), f32)
    for G in range(8):
        for q in range(4):
            gt = G * 4 + q
            wex[32 * q:32 * q + 32, G * 128:(G + 1) * 128] = \
                A_cat[:, gt * 128:(gt + 1) * 128]

    biasT = bias.reshape(NG, 128).T.copy()  # (128, 32)
    return wsq, wrep, wex.astype(ml_dtypes.bfloat16), biasT


def _build_nc():
    import concourse.bacc as bacc
    import concourse.mybir as mybir
    import concourse.tile as tile
    
    dt = mybir.dt.float32
    bt = mybir.dt.bfloat16
    nc = bacc.Bacc()
    x_d = nc.dram_tensor("xT", (D, NS), dt, kind="ExternalInput")
    wsq_d = nc.dram_tensor("wsq", (128, NJ * 128), dt, kind="ExternalInput")
    wrep_d = nc.dram_tensor("wrep", (128, 128), dt, kind="ExternalInput")
    wex_d = nc.dram_tensor("wex", (128, 8 * 128), bt, kind="ExternalInput")
    biasT_d = nc.dram_tensor("biasT", (128, NG), dt, kind="ExternalInput")
    out_d = nc.dram_tensor("out", (T, NJ, BS, 128), dt, kind="ExternalOutput")
    c_d = nc.dram_tensor("c_out", (NJ, BS, 128), dt, kind="ExternalOutput")

    with tile.TileContext(nc) as tc:
        with (
            tc.tile_pool(name="consts", bufs=1) as consts,
            tc.tile_pool(name="state", bufs=1) as state,
            tc.tile_pool(name="xload", bufs=8) as xload,
            tc.tile_pool(name="work", bufs=4) as work,
            tc.tile_pool(name="ps_sq", bufs=2, space="PSUM") as ps_sq,
            tc.tile_pool(name="ps_g", bufs=2, space="PSUM") as ps_g,
        ):
            wsq0 = consts.tile([128, NJ * 128], dt)
            nc.sync.dma_start(out=wsq0, in_=wsq_d[:, :])
            wsq = consts.tile([128, NJ * 128], dt)
            nc.vector.tensor_copy(wsq, wsq0)
            wrep0 = consts.tile([128, 128], dt)
            nc.sync.dma_start(out=wrep0, in_=wrep_d[:, :])
            wrep = consts.tile([128, 128], dt)
            nc.vector.tensor_copy(wrep, wrep0)
            wex0 = consts.tile([128, 8 * 128], bt)
            nc.sync.dma_start(out=wex0, in_=wex_d[:, :])
            wex = consts.tile([128, 8 * 128], bt)
            nc.vector.tensor_copy(wex, wex0)
            biasT = consts.tile([128, NG], dt)
            nc.sync.dma_start(out=biasT, in_=biasT_d[:, :])

            sqiT = consts.tile([128, NS], bt)        # rows 32q+16+a = sqi, t-major cols
            hT_a = state.tile([128, NJ * BS], dt)
            hT_b = state.tile([128, NJ * BS], dt)
            hTs = [hT_a, hT_b]
            cT_a = state.tile([128, NJ * BS], dt)
            cT_b = state.tile([128, NJ * BS], dt)
            cTs = [cT_a, cT_b]
            repl_a = state.tile([128, 4 * BS], bt)
            repl_b = state.tile([128, 4 * BS], bt)
            repls = [repl_a, repl_b]
            nc.vector.memset(hT_a, 0.0)
            nc.vector.memset(hT_b, 0.0)
            nc.vector.memset(cT_a, 0.0)
            nc.vector.memset(cT_b, 0.0)
            for r_ in repls:
                nc.vector.memset(r_, 0.0)

            # ---------------- phase 1: sqiT = Ai_s^T @ xT, t-major ----------
            for sl in range(8):
                xs = xload.tile([128, NJ, 512], dt)
                src = x_d.rearrange("(j p) n -> p j n", p=128)
                nc.sync.dma_start(out=xs, in_=src[:, :, sl * 512:(sl + 1) * 512])
                sq_ps = ps_sq.tile([128, 512], dt, tag="sq1")
                for j in range(NJ):
                    nc.tensor.matmul(sq_ps, wsq[:, j * 128:(j + 1) * 128],
                                     xs[:, j, :],
                                     start=(j == 0), stop=(j == NJ - 1))
                nc.vector.tensor_copy(sqiT[:, sl * 512:(sl + 1) * 512], sq_ps)

            # ---------------- phase 2: recurrence ---------------------------
            for t in range(T):
                # s_hh (16,8) = Ah_s^T h, accumulated over 8 k-tiles
                s_ps = ps_sq.tile([R, BS], dt, tag="sq")
                for j in range(NJ):
                    nc.tensor.matmul(s_ps, wrep[:, j * R:(j + 1) * R],
                                     hT[:, j * BS:(j + 1) * BS],
                                     start=(j == 0), stop=(j == NJ - 1))
                repl = repls[t % 2]
                # block-diag rhs: per q copy [zeros16; sqi16] then s_hh over rows 0:16
                for q in range(4):
                    nc.vector.tensor_copy(
                        repl[32 * q:32 * q + 32, q * BS:(q + 1) * BS],
                        sqiT[32 * q:32 * q + 32, t * BS:(t + 1) * BS])
                for q in range(4):
                    nc.vector.tensor_copy(
                        repl[32 * q:32 * q + R, q * BS:(q + 1) * BS], s_ps)
                g_ps = ps_g.tile([128, NG * BS], dt, tag="gates")
                for G in range(8):
                    nc.tensor.matmul(g_ps[:, G * 32:(G + 1) * 32],
                                     wex[:, G * 128:(G + 1) * 128], repl,
                                     start=True, stop=True)
                gates = work.tile([128, NG * BS], dt, tag="gates_sb")
                # bias add (broadcast over batch) psum -> sbuf
                nc.vector.tensor_add(
                    gates.rearrange("p (g b) -> p g b", b=BS),
                    g_ps.rearrange("p (g b) -> p g b", b=BS),
                    biasT.rearrange("p (g o) -> p g o", o=1).to_broadcast([128, NG, BS]))
                sig = work.tile([128, 24 * BS], dt, tag="sig")
                nc.scalar.activation(sig, gates[:, :24 * BS],
                                     mybir.ActivationFunctionType.Sigmoid)
                tg = work.tile([128, NJ * BS], dt, tag="tg")
                nc.scalar.activation(tg, gates[:, 24 * BS:],
                                     mybir.ActivationFunctionType.Tanh)
                # c = sig_f * c + sig_i * tg
                t1 = work.tile([128, NJ * BS], dt, tag="t1")
                nc.vector.tensor_mul(t1, sig[:, :NJ * BS], tg)
                nc.vector.tensor_mul(cT, sig[:, NJ * BS:2 * NJ * BS], cT)
                nc.vector.tensor_add(cT, cT, t1)
                tc_t = work.tile([128, NJ * BS], dt, tag="tc")
                nc.scalar.activation(tc_t, cT, mybir.ActivationFunctionType.Tanh)
                nc.vector.tensor_mul(hT, sig[:, 2 * NJ * BS:3 * NJ * BS], tc_t)
                # stream h_t to DRAM: out[b, t, j*128+p] = hT[p, j*8+b]
                if "dma" not in SKIP:
                    dst = out_d[t].rearrange("j b p -> p j b")
                    nc.sync.dma_start(out=dst, in_=hT.rearrange("p (j b) -> p j b", b=BS))

            dstc = c_d.rearrange("j b p -> p j b")
            nc.sync.dma_start(out=dstc, in_=cT.rearrange("p (j b) -> p j b", b=BS))
    nc.compile()
    return nc


def kernel(x, ih_g1, ih_g2, ih_g3, ih_g4, ih_g5, ih_g6,
           hh_g1, hh_g2, hh_g3, hh_g4, hh_g5, hh_g6, b_ih, b_hh):
    from concourse.bass_utils import run_bass_kernel_spmd

    x = np.asarray(x, dtype=f32)
    wsq, wrep, wex, biasT = _compose_host(
        [np.asarray(g, f32) for g in (ih_g1, ih_g2, ih_g3, ih_g4, ih_g5, ih_g6)],
        [np.asarray(g, f32) for g in (hh_g1, hh_g2, hh_g3, hh_g4, hh_g5, hh_g6)],
        b_ih, b_hh)

    if "nc" not in _cache:
        _cache["nc"] = _build_nc()
    nc = _cache["nc"]

    in_maps = []
    for c in range(NCORES):
        xs = np.ascontiguousarray(
            x[c * BS:(c + 1) * BS].transpose(2, 1, 0).reshape(D, NS))
        in_maps.append({"xT": xs, "wsq": wsq, "wrep": wrep, "wex": wex,
                        "biasT": biasT})

    trace = bool(int(os.environ.get("KBENCH_TRACE", "0")))
    res = run_bass_kernel_spmd(nc, in_maps, core_ids=list(range(NCORES)),
                               trace=trace)
    if trace:
        print(f"HW exec time: {res.exec_time_ns} ns")
        if res.instructions_and_trace:
            print("trace:", res.instructions_and_trace[1])

    out = np.empty((B, T, H), f32)
    c_fin = np.empty((B, H), f32)
    for c in range(NCORES):
        o = res.results[c]["out"]           # (T, NJ, BS, 128)
        out[c * BS:(c + 1) * BS] = o.transpose(2, 0, 1, 3).reshape(BS, T, H)
        c_fin[c * BS:(c + 1) * BS] = res.results[c]["c_out"].transpose(1, 0, 2).reshape(BS, H)
    h_fin = np.ascontiguousarray(out[:, -1, :])
    return out, (h_fin, c_fin)
